# revision 6
# baseline (speedup 1.0000x reference)
"""Trainium2 Bass kernel for the LeNet C3 dense-conv layer.

Computes out = conv2d_valid(x, K, stride 1) + bias where K is the dense
[16, 6, 5, 5] kernel scattered from the sparse per-branch weights
(w3/w4/w6), x is [128, 6, 256, 256] f32, out is [128, 16, 252, 252] f32.

Strategy (v2):
  - Pure data parallelism: 16 images per NeuronCore across 8 cores.
  - Conv as shift-accumulated banded matmuls into PSUM, fp16 operands.
    Contraction stacks TWO copies of the 10 input rows of a 6-output-row
    block, the second pre-shifted one column, so each matmul covers two
    kernel columns kx: 3 matmuls per image pair instead of 5.
  - v2: the second (shifted) copy is built ON-CHIP by a GpSimd
    partition-offset copy instead of being DMA'd from HBM, halving the
    input HBM traffic (the DMA engines were the bottleneck at ~90%).
    Engine partition writes must start at a 32-aligned partition, so the
    copy lands at partitions 64..123 and rows 60..63 are zeroed (zero
    wall rows there keep the math identical). K = 124 partitions.
  - Matmuls are ordered wall-outer (all 4 image pairs for one wall
    before swapping walls): 3 weight swaps per sub-round instead of 12.
  - PSUM evictions (bias add + fp16 cast fused) alternate between the
    Vector (DVE) and Scalar (Act) engines so neither becomes critical;
    each eviction's output DMA is queued from the engine that made it.
  - fp16 I/O staging: ~5e-4 relative quantization, well under the 2e-2
    gate. Host packs/unpacks layouts.
"""

import numpy as np

# LeNet-5 C3 sparse channel connectivity (from the model definition).
CH3 = np.array([[0, 1, 2], [1, 2, 3], [2, 3, 4], [3, 4, 5], [0, 4, 5], [0, 1, 5]])
CH4 = np.array([[0, 1, 2, 3], [1, 2, 3, 4], [2, 3, 4, 5], [0, 3, 4, 5],
                [0, 1, 4, 5], [0, 1, 2, 5], [0, 1, 3, 4], [1, 2, 4, 5],
                [0, 2, 3, 5]])

B, C, H, W = 128, 6, 256, 256
CO, HO, WO = 16, 252, 252
NCORES = 8
BPC = B // NCORES           # images per core (16)
KH = KW = 5

R = 6                       # output rows per block
HI = R + 4                  # input rows per block (10)
NBLK = HO // R              # 42 blocks
KK = C * HI                 # contraction rows per kx copy (60)
MM = CO * R                 # psum partitions (96)
TW = 4 + BPC * W            # stream width (4100)
TWP = TW + 1                # tile width (+1 col so the on-chip shifted
                            # copy's last used column reads a zero)
KP = 124                    # matmul contraction partitions (60+4gap+60)

_STATE = None  # cached Bass module so repeat kernel() calls skip re-tracing


def _dense_kernel(w3, w4, w6):
    k = np.zeros((CO, C, KH, KW), np.float32)
    k[np.arange(6)[:, None], CH3] = w3
    k[6 + np.arange(9)[:, None], CH4] = w4
    k[15] = w6[0]
    return k


def _band(kd, kx):
    """Banded lhsT [KK, MM] for kernel column kx: row i*6 + c_in,
    column c_out*R + r, value kd[c_out, c_in, i-r, kx]."""
    out = np.zeros((KK, MM), np.float32)
    for ci in range(C):
        for i in range(HI):
            for r in range(R):
                ky = i - r
                if 0 <= ky < KH:
                    out[i * C + ci, np.arange(CO) * R + r] = kd[:, ci, ky, kx]
    return out


def _build_module():
    import concourse.bacc as bacc
    import concourse.mybir as mybir
    from concourse.tile import TileContext

    f32 = mybir.dt.float32
    f16 = mybir.dt.float16
    IDENT = mybir.ActivationFunctionType.Identity

    nc = bacc.Bacc(None)
    # Flat per-block input tiles: 60 data rows + 4 zero rows (gap).
    x_d = nc.dram_tensor("x", [NBLK, 64, TWP], f16, kind="ExternalInput")
    # Block 0 fully stacked on the host: the first matmuls skip the
    # on-chip copy dependency, removing the startup PE gap.
    x0_d = nc.dram_tensor("x0", [KP, TWP], f16, kind="ExternalInput")
    # wall: [124, 3*96] = [B(0);0;B(1)] | [B(2);0;B(3)] | [B(4);0;0]
    wall_d = nc.dram_tensor("wall", [KP, 3 * MM], f16, kind="ExternalInput")
    b1_d = nc.dram_tensor("b1", [MM, 1], f32, kind="ExternalInput")
    # o8[oc, c, h, j*252 + w] = out[8*oc + j, c, h, w]  (host un-packs).
    o_d = nc.dram_tensor("o", [2, CO, HO, 8 * WO], f16, kind="ExternalOutput")

    with TileContext(nc) as tc:
        with (
            tc.tile_pool(name="wpool", bufs=1) as wp,
            tc.tile_pool(name="inpool", bufs=4) as ip,
            tc.tile_pool(name="outpool", bufs=6) as op,
            tc.tile_pool(name="pspool", bufs=2, space="PSUM") as pp,
        ):
            wall_t = wp.tile([KP, 3 * MM], f16)
            nc.sync.dma_start(wall_t[:], wall_d[:])
            b1_t = wp.tile([MM, 1], f32)
            nc.sync.dma_start(b1_t[:], b1_d[:])

            # Prime the constant tiles on their consuming engine classes so
            # steady-state instructions carry few semaphore waits.
            prime_ps = pp.tile([MM, 192], f32, tag="ps")
            nc.tensor.matmul(prime_ps[:], wall_t[:, 0:MM], wall_t[:, 0:192],
                             start=True, stop=True)
            prime_b = op.tile([MM, 1], f16, tag="out")
            nc.vector.tensor_scalar_add(prime_b[:], b1_t[:], 0.0)
            prime_a = op.tile([MM, 1], f16, tag="out")
            nc.scalar.activation(prime_a[:], b1_t[:], IDENT, bias=0.0, scale=1.0)

            for g in range(NBLK):
                h0 = R * g
                it = ip.tile([KP, TWP], f16, tag="in")
                if g == 0:
                    nc.sync.dma_start(it[:, :], x0_d[:])
                else:
                    nc.sync.dma_start(it[0:64, :], x_d[g])
                    # On-chip shifted duplicate: rows 64..123 = rows 0..59
                    # advanced one column (column TWP-1 of the source is
                    # the host-zeroed pad, so every copied column is
                    # defined). DVE: fp16 runs 2 elem/lane/cycle (~2.1us).
                    nc.vector.tensor_copy(it[64:KP, 0:TWP - 1],
                                          it[0:60, 1:TWP])

                for sr in range(2):             # 8-image sub-rounds
                    ps = pp.tile([MM, 8, 256], f32, tag="ps")
                    for w in range(3):          # wall-outer: 3 swaps/subround
                        for grp in range(4):
                            b = 2048 * sr + 512 * grp + 2 * w
                            nc.tensor.matmul(ps[:, 2 * grp:2 * grp + 2, :],
                                             wall_t[:, MM * w:MM * (w + 1)],
                                             it[:, b:b + 512],
                                             start=(w == 0), stop=(w == 2))

                    ot = op.tile([MM, 8 * WO], f16, tag="out")
                    ov = ot[:].rearrange("p (j w) -> p j w", j=8)
                    # All evictions on Act: DVE then only runs copies and
                    # can work ahead of the PE without head-of-line stalls.
                    if g == NBLK - 1 and sr == 1:
                        # Tail: evict/DMA in 4 pipelined chunks so the
                        # final output DMA starts as early as possible.
                        for jj in range(4):
                            nc.scalar.activation(
                                ov[:, 2 * jj:2 * jj + 2, :],
                                ps[:, 2 * jj:2 * jj + 2, 4:4 + WO], IDENT,
                                bias=b1_t[:, 0:1], scale=1.0)
                            nc.scalar.dma_start(
                                o_d[sr, :, h0:h0 + R,
                                    2 * WO * jj:2 * WO * (jj + 1)],
                                ot[:, 2 * WO * jj:2 * WO * (jj + 1)])
                    else:
                        nc.scalar.activation(ov, ps[:, :, 4:4 + WO], IDENT,
                                             bias=b1_t[:, 0:1], scale=1.0)
                        nc.scalar.dma_start(o_d[sr, :, h0:h0 + R, :], ot[:])
    nc.compile()
    return nc


def _get_module():
    global _STATE
    if _STATE is None:
        _STATE = _build_module()
    return _STATE


def kernel(x, w3, b3, w4, b4, w6, b6):
    from concourse.bass_utils import run_bass_kernel_spmd

    x = np.asarray(x, np.float32)
    kd = _dense_kernel(np.asarray(w3, np.float32), np.asarray(w4, np.float32),
                       np.asarray(w6, np.float32))
    bias = np.concatenate([np.asarray(b3, np.float32),
                           np.asarray(b4, np.float32),
                           np.asarray(b6, np.float32)])

    wall = np.zeros((KP, 3 * MM), np.float32)
    for w in range(3):
        wall[0:KK, MM * w:MM * (w + 1)] = _band(kd, 2 * w)
        if w < 2:
            wall[64:KP, MM * w:MM * (w + 1)] = _band(kd, 2 * w + 1)
    wall = wall.astype(np.float16)
    b1 = np.repeat(bias, R).astype(np.float32).reshape(MM, 1)

    nc = _get_module()
    x16 = x.astype(np.float16)
    in_maps = []
    for cr in range(NCORES):
        xs = x16[cr * BPC:(cr + 1) * BPC]
        # rows[(h, c), j*256 + w] = x[j, c, h, w]
        rows = np.ascontiguousarray(
            xs.transpose(2, 1, 0, 3)).reshape(H * C, BPC * W)
        xflat = np.zeros((NBLK, 64, TWP), np.float16)
        for g in range(NBLK):
            xflat[g, 0:KK, 4:4 + BPC * W] = rows[R * C * g: R * C * g + KK]
        x0 = np.zeros((KP, TWP), np.float16)
        x0[0:64] = xflat[0]
        x0[64:KP, 0:TWP - 1] = xflat[0][0:60, 1:TWP]
        in_maps.append({"x": xflat, "x0": x0, "wall": wall, "b1": b1})
    res = run_bass_kernel_spmd(nc, in_maps, core_ids=list(range(NCORES)))
    global LAST_RESULT
    LAST_RESULT = res

    out = np.empty((B, CO, HO, WO), np.float32)
    for cr in range(NCORES):
        o8 = res.results[cr]["o"].astype(np.float32).reshape(2, CO, HO, 8, WO)
        out[cr * BPC:(cr + 1) * BPC] = (
            o8.transpose(0, 3, 1, 2, 4).reshape(BPC, CO, HO, WO)
        )
    return out


LAST_RESULT = None


# revision 7
# speedup vs baseline: 1.0484x; 1.0484x over previous
"""Trainium2 Bass kernel for the LeNet C3 dense-conv layer.

Computes out = conv2d_valid(x, K, stride 1) + bias where K is the dense
[16, 6, 5, 5] kernel scattered from the sparse per-branch weights
(w3/w4/w6), x is [128, 6, 256, 256] f32, out is [128, 16, 252, 252] f32.

Strategy (v2):
  - Pure data parallelism: 16 images per NeuronCore across 8 cores.
  - Conv as shift-accumulated banded matmuls into PSUM, fp16 operands.
    Contraction stacks TWO copies of the 10 input rows of a 6-output-row
    block, the second pre-shifted one column, so each matmul covers two
    kernel columns kx: 3 matmuls per image pair instead of 5.
  - v2: the second (shifted) copy is built ON-CHIP by a GpSimd
    partition-offset copy instead of being DMA'd from HBM, halving the
    input HBM traffic (the DMA engines were the bottleneck at ~90%).
    Engine partition writes must start at a 32-aligned partition, so the
    copy lands at partitions 64..123 and rows 60..63 are zeroed (zero
    wall rows there keep the math identical). K = 124 partitions.
  - Matmuls are ordered wall-outer (all 4 image pairs for one wall
    before swapping walls): 3 weight swaps per sub-round instead of 12.
  - PSUM evictions (bias add + fp16 cast fused) alternate between the
    Vector (DVE) and Scalar (Act) engines so neither becomes critical;
    each eviction's output DMA is queued from the engine that made it.
  - fp16 I/O staging: ~5e-4 relative quantization, well under the 2e-2
    gate. Host packs/unpacks layouts.
"""

import numpy as np

# LeNet-5 C3 sparse channel connectivity (from the model definition).
CH3 = np.array([[0, 1, 2], [1, 2, 3], [2, 3, 4], [3, 4, 5], [0, 4, 5], [0, 1, 5]])
CH4 = np.array([[0, 1, 2, 3], [1, 2, 3, 4], [2, 3, 4, 5], [0, 3, 4, 5],
                [0, 1, 4, 5], [0, 1, 2, 5], [0, 1, 3, 4], [1, 2, 4, 5],
                [0, 2, 3, 5]])

B, C, H, W = 128, 6, 256, 256
CO, HO, WO = 16, 252, 252
NCORES = 8
BPC = B // NCORES           # images per core (16)
KH = KW = 5

R = 6                       # output rows per block
HI = R + 4                  # input rows per block (10)
NBLK = HO // R              # 42 blocks
KK = C * HI                 # contraction rows per kx copy (60)
MM = CO * R                 # psum partitions (96)
TW = 4 + BPC * W            # stream width (4100)
TWP = TW + 1                # tile width (+1 col so the on-chip shifted
                            # copy's last used column reads a zero)
KP = 124                    # matmul contraction partitions (60+4gap+60)

_STATE = None  # cached Bass module so repeat kernel() calls skip re-tracing


def _dense_kernel(w3, w4, w6):
    k = np.zeros((CO, C, KH, KW), np.float32)
    k[np.arange(6)[:, None], CH3] = w3
    k[6 + np.arange(9)[:, None], CH4] = w4
    k[15] = w6[0]
    return k


def _band(kd, kx):
    """Banded lhsT [KK, MM] for kernel column kx: row i*6 + c_in,
    column c_out*R + r, value kd[c_out, c_in, i-r, kx]."""
    out = np.zeros((KK, MM), np.float32)
    for ci in range(C):
        for i in range(HI):
            for r in range(R):
                ky = i - r
                if 0 <= ky < KH:
                    out[i * C + ci, np.arange(CO) * R + r] = kd[:, ci, ky, kx]
    return out


def _build_module():
    import concourse.bacc as bacc
    import concourse.mybir as mybir
    from concourse.tile import TileContext

    f32 = mybir.dt.float32
    f16 = mybir.dt.float16
    IDENT = mybir.ActivationFunctionType.Identity

    nc = bacc.Bacc(None)
    # Flat per-block input tiles: 60 data rows + 4 zero rows (gap).
    x_d = nc.dram_tensor("x", [NBLK, 64, TWP], f16, kind="ExternalInput")
    # wall: [124, 3*96] = [B(0);0;B(1)] | [B(2);0;B(3)] | [B(4);0;0]
    wall_d = nc.dram_tensor("wall", [KP, 3 * MM], f16, kind="ExternalInput")
    b1_d = nc.dram_tensor("b1", [MM, 1], f32, kind="ExternalInput")
    # o8[oc, c, h, j*252 + w] = out[8*oc + j, c, h, w]  (host un-packs).
    o_d = nc.dram_tensor("o", [2, CO, HO, 8 * WO], f16, kind="ExternalOutput")

    with TileContext(nc) as tc:
        with (
            tc.tile_pool(name="wpool", bufs=1) as wp,
            tc.tile_pool(name="inpool", bufs=4) as ip,
            tc.tile_pool(name="outpool", bufs=6) as op,
            tc.tile_pool(name="pspool", bufs=2, space="PSUM") as pp,
        ):
            wall_t = wp.tile([KP, 3 * MM], f16)
            nc.sync.dma_start(wall_t[:], wall_d[:])
            b1_t = wp.tile([MM, 1], f32)
            nc.sync.dma_start(b1_t[:], b1_d[:])

            # Prime the constant tiles on their consuming engine classes so
            # steady-state instructions carry few semaphore waits.
            prime_ps = pp.tile([MM, 192], f32, tag="ps")
            nc.tensor.matmul(prime_ps[:], wall_t[:, 0:MM], wall_t[:, 0:192],
                             start=True, stop=True)
            prime_b = op.tile([MM, 1], f16, tag="out")
            nc.vector.tensor_scalar_add(prime_b[:], b1_t[:], 0.0)
            prime_a = op.tile([MM, 1], f16, tag="out")
            nc.scalar.activation(prime_a[:], b1_t[:], IDENT, bias=0.0, scale=1.0)

            for g in range(NBLK):
                h0 = R * g
                it = ip.tile([KP, TWP], f16, tag="in")
                nc.sync.dma_start(it[0:64, :], x_d[g])
                # On-chip shifted duplicate: rows 64..123 = rows 0..59
                # advanced one column (column TWP-1 of the source is the
                # host-zeroed pad, so every copied column is defined).
                # DVE: fp16 runs at 2 elem/lane/cycle (~2.1us per block).
                nc.vector.tensor_copy(it[64:KP, 0:TWP - 1],
                                      it[0:60, 1:TWP])

                for sr in range(2):             # 8-image sub-rounds
                    ps = pp.tile([MM, 8, 256], f32, tag="ps")
                    for w in range(3):          # wall-outer: 3 swaps/subround
                        for grp in range(4):
                            b = 2048 * sr + 512 * grp + 2 * w
                            nc.tensor.matmul(ps[:, 2 * grp:2 * grp + 2, :],
                                             wall_t[:, MM * w:MM * (w + 1)],
                                             it[:, b:b + 512],
                                             start=(w == 0), stop=(w == 2))

                    ot = op.tile([MM, 8 * WO], f16, tag="out")
                    ov = ot[:].rearrange("p (j w) -> p j w", j=8)
                    # All evictions on Act: DVE then only runs copies and
                    # can work ahead of the PE without head-of-line stalls.
                    if g == NBLK - 1 and sr == 1:
                        # Tail: evict/DMA in 4 pipelined chunks so the
                        # final output DMA starts as early as possible.
                        for jj in range(4):
                            nc.scalar.activation(
                                ov[:, 2 * jj:2 * jj + 2, :],
                                ps[:, 2 * jj:2 * jj + 2, 4:4 + WO], IDENT,
                                bias=b1_t[:, 0:1], scale=1.0)
                            nc.scalar.dma_start(
                                o_d[sr, :, h0:h0 + R,
                                    2 * WO * jj:2 * WO * (jj + 1)],
                                ot[:, 2 * WO * jj:2 * WO * (jj + 1)])
                    else:
                        nc.scalar.activation(ov, ps[:, :, 4:4 + WO], IDENT,
                                             bias=b1_t[:, 0:1], scale=1.0)
                        nc.scalar.dma_start(o_d[sr, :, h0:h0 + R, :], ot[:])
    nc.compile()
    return nc


def _get_module():
    global _STATE
    if _STATE is None:
        _STATE = _build_module()
    return _STATE


def kernel(x, w3, b3, w4, b4, w6, b6):
    from concourse.bass_utils import run_bass_kernel_spmd

    x = np.asarray(x, np.float32)
    kd = _dense_kernel(np.asarray(w3, np.float32), np.asarray(w4, np.float32),
                       np.asarray(w6, np.float32))
    bias = np.concatenate([np.asarray(b3, np.float32),
                           np.asarray(b4, np.float32),
                           np.asarray(b6, np.float32)])

    wall = np.zeros((KP, 3 * MM), np.float32)
    for w in range(3):
        wall[0:KK, MM * w:MM * (w + 1)] = _band(kd, 2 * w)
        if w < 2:
            wall[64:KP, MM * w:MM * (w + 1)] = _band(kd, 2 * w + 1)
    wall = wall.astype(np.float16)
    b1 = np.repeat(bias, R).astype(np.float32).reshape(MM, 1)

    nc = _get_module()
    x16 = x.astype(np.float16)
    in_maps = []
    for cr in range(NCORES):
        xs = x16[cr * BPC:(cr + 1) * BPC]
        # rows[(h, c), j*256 + w] = x[j, c, h, w]
        rows = np.ascontiguousarray(
            xs.transpose(2, 1, 0, 3)).reshape(H * C, BPC * W)
        xflat = np.zeros((NBLK, 64, TWP), np.float16)
        for g in range(NBLK):
            xflat[g, 0:KK, 4:4 + BPC * W] = rows[R * C * g: R * C * g + KK]
        in_maps.append({"x": xflat, "wall": wall, "b1": b1})
    res = run_bass_kernel_spmd(nc, in_maps, core_ids=list(range(NCORES)))
    global LAST_RESULT
    LAST_RESULT = res

    out = np.empty((B, CO, HO, WO), np.float32)
    for cr in range(NCORES):
        o8 = res.results[cr]["o"].astype(np.float32).reshape(2, CO, HO, 8, WO)
        out[cr * BPC:(cr + 1) * BPC] = (
            o8.transpose(0, 3, 1, 2, 4).reshape(BPC, CO, HO, WO)
        )
    return out


LAST_RESULT = None
